# revision 6
# baseline (speedup 1.0000x reference)
"""Soft Needleman-Wunsch aligner (forward value DP + gradient decode) on TRN2.

Strategy: batch-parallel over 8 NeuronCores (one [512,512] problem per core).
Per core, the DP runs in exp-space: W[i,j] = exp(V[i,j] - V0[i,j]) where V0 is
the DP table for constant scores (th0=ln2, a0=-ln2), host-precomputed in f64.
This turns the log-sum-exp recurrence into a linear 3-term recurrence
  W[i,j] = cl*W[i,j-1] + cu*W[i-1,j] + cd*W[i-1,j-1]
evaluated in a skewed SBUF layout (cell (i,j) -> partition (i-1)%128, column
(j-1)+(i-1)*C) where each "band" of C columns advances the wavefront one row
on every partition at once.  The cross-partition (up/diag) term is produced
by a cyclic-rotation matmul on the PE engine; the in-partition (left) term is
a tensor_tensor_scan on DVE.  The backward/gradient pass is the same
machinery run with a descending scan and down-rotation.
"""
import numpy as np

NEG = -1e9
MASK = -1e4
C = 12
TH0 = float(np.log(2.0))
A0 = float(-np.log(2.0))

_F32 = None  # set lazily (mybir.dt.float32)


# ---------------------------------------------------------------- host tables
def _v0_table(N, M):
    V = np.full((N + 1, M + 1), NEG, np.float64)
    V[0, 0] = 0.0
    for k in range(2, N + M + 1):
        i = np.arange(max(1, k - M), min(N, k - 1) + 1)
        j = k - i
        st = np.stack([A0 + V[i - 1, j], V[i - 1, j - 1], A0 + V[i, j - 1]])
        m = st.max(axis=0)
        V[i, j] = TH0 + m + np.log(np.exp(st - m).sum(axis=0))
    return V


def _build_tables(N, M):
    """Log-space E-tables (V0-diff + boundary masks), f32 [N, M]."""
    V0 = _v0_table(N, M)
    El = (V0[1:, :-1] - V0[1:, 1:]).copy()
    Eu = (V0[:-1, 1:] - V0[1:, 1:]).copy()
    Ed = (V0[:-1, :-1] - V0[1:, 1:]).copy()
    El[:, 0] = MASK
    Eu[0, :] = MASK
    Ed[0, 1:] = MASK
    Ed[1:, 0] = MASK  # Ed[0,0] stays real: seeds W[1,1]
    Fu = np.full((N, M), MASK, np.float64)
    Fd = np.full((N, M), MASK, np.float64)
    Fl = np.full((N, M), MASK, np.float64)
    Fu[:-1, :] = V0[1:-1, 1:] - V0[2:, 1:]
    Fd[:-1, :-1] = V0[1:-1, 1:-1] - V0[2:, 2:]
    Fl[:, :-1] = V0[1:, 1:-1] - V0[1:, 2:]
    f = np.float32
    return El.astype(f), Eu.astype(f), Ed.astype(f), Fu.astype(f), Fl.astype(f), Fd.astype(f)


def _consts():
    P = 128
    rot = np.zeros((P, P), np.float32)
    rot[np.arange(P), (np.arange(P) + 1) % P] = 1.0    # out[m] = in[m-1 mod P]
    rot2 = rot.T.copy()                                 # out[m] = in[m+1 mod P]
    e01 = np.zeros((P, P), np.float32)
    e01[0, P - 1] = 1.0                                 # out[127] += in[0]
    idm = np.eye(P, dtype=np.float32)
    return rot, rot2, e01, idm


# ---------------------------------------------------------------- bass builder
def build_nc(N, D):
    import concourse.bacc as bacc
    import concourse.bass as bass
    import concourse.mybir as mybir
    import concourse.tile as tile

    F32 = mybir.dt.float32
    AF = mybir.ActivationFunctionType
    OP = mybir.AluOpType
    P = 128
    M = N
    nw = N // P                      # row wraps
    nd = D // P                      # contraction chunks
    U = (N - 1) * (C + 1) + 1        # max skew col + 1
    S = (U + C - 1) // C             # number of bands
    SC = S * C
    PADW = C + 1
    WWID = PADW + SC                 # W buffer width
    ZWID = SC + C + 1                # Zs buffer width
    KWID = SC                        # coeff buffer width
    WRAPC = P * C                    # skew col offset per wrap

    nc = bacc.Bacc('TRN2', target_bir_lowering=False, debug=False)

    din = {}
    for nm in ('zx', 'zy', 'gx', 'gy'):
        din[nm] = nc.dram_tensor(nm, (N, D), F32, kind='ExternalInput')
    for nm in ('el', 'eu', 'ed', 'fu', 'fl', 'fd'):
        din[nm] = nc.dram_tensor(nm, (N, M), F32, kind='ExternalInput')
    for nm in ('rotm', 'rot2m', 'e01m', 'idm'):
        din[nm] = nc.dram_tensor(nm, (P, P), F32, kind='ExternalInput')
    o_aln = nc.dram_tensor('o_aln', (N, M), F32, kind='ExternalOutput')
    o_th = nc.dram_tensor('o_th', (N, M), F32, kind='ExternalOutput')
    o_a = nc.dram_tensor('o_a', (N, M), F32, kind='ExternalOutput')

    def skew_dst(base_ap, k):
        (ps, _), _ = base_ap.ap
        return bass.AP(base_ap.tensor, base_ap.offset + k * WRAPC,
                       [[ps + C, P], [1, M]])

    def rev_free(ap2):
        (ps, p), (fs, w) = ap2.ap
        assert fs == 1
        return bass.AP(ap2.tensor, ap2.offset + w - 1, [[ps, p], [-1, w]])

    with tile.TileContext(nc) as tc:
        with tc.tile_pool(name='big', bufs=1) as big:
            W_t = big.tile([P, WWID], F32, tag='W')
            Zs_t = big.tile([P, ZWID], F32, tag='Zs')
            CL_t = big.tile([P, KWID], F32, tag='CL')
            CU_t = big.tile([P, KWID], F32, tag='CU')
            CD_t = big.tile([P, KWID], F32, tag='CD')
            th_nat = [big.tile([P, M], F32, tag=f'th{k}', name=f'th{k}') for k in range(nw)]
            a_nat = [big.tile([P, M], F32, tag=f'a{k}', name=f'a{k}') for k in range(nw)]
            rot_t = big.tile([P, P], F32, tag='rot')
            rot2_t = big.tile([P, P], F32, tag='rot2')
            e01_t = big.tile([P, P], F32, tag='e01')
            id_t = big.tile([P, P], F32, tag='idm')
            for t, nm in ((rot_t, 'rotm'), (rot2_t, 'rot2m'), (e01_t, 'e01m'), (id_t, 'idm')):
                nc.sync.dma_start(t[:, :], din[nm][:, :])

            # ---------------- phase 1: scores th = softplus(zx@zy^T), A = logsig(gx@gy^T)
            with tc.tile_pool(name='mm', bufs=1) as mm, \
                 tc.psum_pool(name='mmp', bufs=2) as mmp:
                for pi, (xn, yn) in enumerate((('zx', 'zy'), ('gx', 'gy'))):
                    xT = [mm.tile([P, N], F32, tag=f'xT{dc}', name=f'xT{dc}') for dc in range(nd)]
                    yT = [mm.tile([P, M], F32, tag=f'yT{dc}', name=f'yT{dc}') for dc in range(nd)]
                    for src, dstT in ((xn, xT), (yn, yT)):
                        for k in range(nw):
                            nat = mm.tile([P, D], F32, tag='nat_ld')
                            nc.sync.dma_start(nat[:, :], din[src][k * P:(k + 1) * P, :])
                            for dc in range(nd):
                                pst = mmp.tile([P, P], F32, tag='tp')
                                nc.tensor.transpose(pst[:, :], nat[:, dc * P:(dc + 1) * P], id_t[:, :])
                                nc.vector.tensor_copy(dstT[dc][:, k * P:(k + 1) * P], pst[:, :])
                    for rb in range(nw):
                        ps = mmp.tile([P, M], F32, tag='score')
                        for dc in range(nd):
                            nc.tensor.matmul(ps[:, :], xT[dc][:, rb * P:(rb + 1) * P], yT[dc][:, :],
                                             start=(dc == 0), stop=(dc == nd - 1))
                        e1 = mm.tile([P, M], F32, tag='e1')
                        if pi == 0:
                            nc.scalar.activation(e1[:, :], ps[:, :], AF.Exp)
                            nc.vector.tensor_scalar_add(e1[:, :], e1[:, :], 1.0)
                            nc.scalar.activation(th_nat[rb][:, :], e1[:, :], AF.Ln)
                            nc.sync.dma_start(o_th[rb * P:(rb + 1) * P, :], th_nat[rb][:, :])
                        else:
                            nc.scalar.activation(e1[:, :], ps[:, :], AF.Exp, scale=-1.0)
                            nc.vector.tensor_scalar_add(e1[:, :], e1[:, :], 1.0)
                            nc.scalar.activation(e1[:, :], e1[:, :], AF.Ln)
                            nc.scalar.activation(a_nat[rb][:, :], e1[:, :], AF.Copy, scale=-1.0)
                            nc.sync.dma_start(o_a[rb * P:(rb + 1) * P, :], a_nat[rb][:, :])

            # ---------------- phase 2: forward coefficient tables (skewed)
            nc.vector.memset(W_t[:, :], 0.0)
            for t in (CL_t, CU_t, CD_t):
                nc.vector.memset(t[:, :], 0.0)
            with tc.tile_pool(name='cf', bufs=2) as cf:
                for k in range(nw):
                    s1 = cf.tile([P, M], F32, tag='s1')
                    nc.vector.tensor_tensor(s1[:, :], th_nat[k][:, :], a_nat[k][:, :], op=OP.add)
                    for tbl, dst, base in (('el', CL_t, 's1'), ('eu', CU_t, 's1'), ('ed', CD_t, 'th')):
                        ew = cf.tile([P, M], F32, tag='ew')
                        nc.sync.dma_start(ew[:, :], din[tbl][k * P:(k + 1) * P, :])
                        arg = cf.tile([P, M], F32, tag='arg')
                        bsrc = s1 if base == 's1' else th_nat[k]
                        nc.vector.tensor_tensor(arg[:, :], bsrc[:, :], ew[:, :], op=OP.add)
                        cn = cf.tile([P, M], F32, tag='cn')
                        nc.scalar.activation(cn[:, :], arg[:, :], AF.Exp)
                        nc.sync.dma_start(skew_dst(dst[:, :], k), cn[:, :])

            # ---------------- phase 3: forward DP
            with tc.tile_pool(name='dp', bufs=4) as dp, \
                 tc.psum_pool(name='dpp', bufs=4) as dpp:
                for s in range(S):
                    lo = s * C
                    win = W_t[:, PADW + lo - C - 1:PADW + lo]
                    ps = dpp.tile([P, C + 1], F32, tag='rot')
                    nc.tensor.matmul(ps[:, :], rot_t[:, :], win, start=True, stop=True)
                    t1 = dp.tile([P, C], F32, tag='t1')
                    nc.vector.tensor_tensor(t1[:, :], CU_t[:, lo:lo + C], ps[:, 1:C + 1], op=OP.mult)
                    t2 = dp.tile([P, C], F32, tag='t2')
                    nc.vector.tensor_tensor(t2[:, :], CD_t[:, lo:lo + C], ps[:, 0:C], op=OP.mult)
                    d1 = dp.tile([P, C], F32, tag='d1')
                    nc.vector.tensor_tensor(d1[:, :], t1[:, :], t2[:, :], op=OP.add)
                    if s == 0:
                        nc.vector.tensor_copy(d1[0:1, 0:1], CD_t[0:1, 0:1])
                    nc.vector.tensor_tensor_scan(
                        W_t[:, PADW + lo:PADW + lo + C], CL_t[:, lo:lo + C], d1[:, :],
                        W_t[:, PADW + lo - 1:PADW + lo], OP.mult, OP.add)

            # ---------------- phase 4: backward coefficient tables (overwrite CL/CU/CD)
            for t in (CL_t, CU_t, CD_t):
                nc.vector.memset(t[:, :], 0.0)
            # CL <- Kl, CU <- Ku, CD <- Kd
            with tc.tile_pool(name='cb', bufs=1) as cb, \
                 tc.psum_pool(name='cbp', bufs=2) as cbp:
                for k in range(nw):
                    s1 = cb.tile([P, M], F32, tag='s1')
                    nc.vector.tensor_tensor(s1[:, :], th_nat[k][:, :], a_nat[k][:, :], op=OP.add)
                    # row-shifted s1 and th via down-rotation (+ next-wrap seam fix)
                    ps_s1 = cbp.tile([P, M], F32, tag='ps_s1')
                    ps_th = cbp.tile([P, M], F32, tag='ps_th')
                    last = (k == nw - 1)
                    if last:
                        nc.tensor.matmul(ps_s1[:, :], rot2_t[:, :], s1[:, :], start=True, stop=True)
                        nc.tensor.matmul(ps_th[:, :], rot2_t[:, :], th_nat[k][:, :], start=True, stop=True)
                    else:
                        s1n = cb.tile([P, M], F32, tag='s1n')
                        nc.vector.tensor_tensor(s1n[:, :], th_nat[k + 1][:, :], a_nat[k + 1][:, :], op=OP.add)
                        dif = cb.tile([P, M], F32, tag='dif')
                        nc.vector.tensor_tensor(dif[:, :], s1n[:, :], s1[:, :], op=OP.subtract)
                        nc.tensor.matmul(ps_s1[:, :], rot2_t[:, :], s1[:, :], start=True, stop=False)
                        nc.tensor.matmul(ps_s1[:, :], e01_t[:, :], dif[:, :], start=False, stop=True)
                        dift = cb.tile([P, M], F32, tag='dift')
                        nc.vector.tensor_tensor(dift[:, :], th_nat[k + 1][:, :], th_nat[k][:, :], op=OP.subtract)
                        nc.tensor.matmul(ps_th[:, :], rot2_t[:, :], th_nat[k][:, :], start=True, stop=False)
                        nc.tensor.matmul(ps_th[:, :], e01_t[:, :], dift[:, :], start=False, stop=True)
                    # Ku = exp(s1_rowshift + Fu)
                    fw = cb.tile([P, M], F32, tag='fw')
                    nc.sync.dma_start(fw[:, :], din['fu'][k * P:(k + 1) * P, :])
                    arg = cb.tile([P, M], F32, tag='argu')
                    nc.vector.tensor_tensor(arg[:, :], ps_s1[:, :], fw[:, :], op=OP.add)
                    cn = cb.tile([P, M], F32, tag='cnu')
                    nc.scalar.activation(cn[:, :], arg[:, :], AF.Exp)
                    nc.sync.dma_start(skew_dst(CU_t[:, :], k), cn[:, :])
                    # Kd = exp(th_rowshift[:, c+1] + Fd)
                    fw2 = cb.tile([P, M], F32, tag='fw2')
                    nc.sync.dma_start(fw2[:, :], din['fd'][k * P:(k + 1) * P, :])
                    arg2 = cb.tile([P, M], F32, tag='argd')
                    nc.vector.memset(arg2[:, M - 1:M], MASK)
                    nc.vector.tensor_tensor(arg2[:, 0:M - 1], ps_th[:, 1:M], fw2[:, 0:M - 1], op=OP.add)
                    cn2 = cb.tile([P, M], F32, tag='cnd')
                    nc.scalar.activation(cn2[:, :], arg2[:, :], AF.Exp)
                    nc.sync.dma_start(skew_dst(CD_t[:, :], k), cn2[:, :])
                    # Kl = exp(s1[:, c+1] + Fl)
                    fw3 = cb.tile([P, M], F32, tag='fw3')
                    nc.sync.dma_start(fw3[:, :], din['fl'][k * P:(k + 1) * P, :])
                    arg3 = cb.tile([P, M], F32, tag='argl')
                    nc.vector.memset(arg3[:, M - 1:M], MASK)
                    nc.vector.tensor_tensor(arg3[:, 0:M - 1], s1[:, 1:M], fw3[:, 0:M - 1], op=OP.add)
                    cn3 = cb.tile([P, M], F32, tag='cnl')
                    nc.scalar.activation(cn3[:, :], arg3[:, :], AF.Exp)
                    nc.sync.dma_start(skew_dst(CL_t[:, :], k), cn3[:, :])

            # ---------------- phase 5: backward DP (descending)
            nc.vector.memset(Zs_t[:, :], 0.0)
            with tc.tile_pool(name='dp2', bufs=4) as dp2, \
                 tc.psum_pool(name='dpp2', bufs=4) as dpp2:
                rt1 = dp2.tile([1, 1], F32, tag='rt1')
                rt2 = dp2.tile([1, 1], F32, tag='rt2')
                nc.sync.dma_start(rt1[0:1, 0:1], W_t[P - 1:P, PADW + U - 1:PADW + U])
                nc.vector.reciprocal(rt2[0:1, 0:1], rt1[0:1, 0:1])
                seed_col = (U - 1) - (S - 1) * C
                for s in range(S - 1, -1, -1):
                    lo = s * C
                    win = Zs_t[:, lo + C:lo + 2 * C + 1]
                    ps = dpp2.tile([P, C + 1], F32, tag='rot')
                    nc.tensor.matmul(ps[:, :], rot2_t[:, :], win, start=True, stop=True)
                    t1 = dp2.tile([P, C], F32, tag='t1')
                    nc.vector.tensor_tensor(t1[:, :], CU_t[:, lo:lo + C], ps[:, 0:C], op=OP.mult)
                    t2 = dp2.tile([P, C], F32, tag='t2')
                    nc.vector.tensor_tensor(t2[:, :], CD_t[:, lo:lo + C], ps[:, 1:C + 1], op=OP.mult)
                    d1 = dp2.tile([P, C], F32, tag='d1')
                    nc.vector.tensor_tensor(d1[:, :], t1[:, :], t2[:, :], op=OP.add)
                    if s == S - 1:
                        nc.sync.dma_start(d1[P - 1:P, seed_col:seed_col + 1], rt2[0:1, 0:1])
                    nc.vector.tensor_tensor_scan(
                        rev_free(Zs_t[:, lo:lo + C]), rev_free(CL_t[:, lo:lo + C]),
                        rev_free(d1[:, :]), Zs_t[:, lo + C:lo + C + 1], OP.mult, OP.add)

            # ---------------- phase 6: aln = Zs * W, de-skew out
            nc.vector.tensor_tensor(Zs_t[:, 0:SC], Zs_t[:, 0:SC], W_t[:, PADW:PADW + SC], op=OP.mult)
            for k in range(nw):
                (zps, _), _ = Zs_t[:, :].ap
                src = bass.AP(Zs_t[:, :].tensor, Zs_t[:, :].offset + k * WRAPC,
                              [[zps + C, P], [1, M]])
                nc.sync.dma_start(o_aln[k * P:(k + 1) * P, :], src)

    nc.compile()
    return nc


_NC_CACHE = {}
TRACE = False
LAST_RESULT = None


def kernel(zx, zy, gx, gy):
    from concourse.bass_utils import run_bass_kernel_spmd

    B, N, D = zx.shape
    key = (N, D)
    if key not in _NC_CACHE:
        _NC_CACHE[key] = build_nc(N, D)
    nc = _NC_CACHE[key]

    El, Eu, Ed, Fu, Fl, Fd = _build_tables(N, N)
    rot, rot2, e01, idm = _consts()
    consts = {'el': El, 'eu': Eu, 'ed': Ed, 'fu': Fu, 'fl': Fl, 'fd': Fd,
              'rotm': rot, 'rot2m': rot2, 'e01m': e01, 'idm': idm}
    in_maps = [dict(zx=np.ascontiguousarray(zx[b]), zy=np.ascontiguousarray(zy[b]),
                    gx=np.ascontiguousarray(gx[b]), gy=np.ascontiguousarray(gy[b]),
                    **consts) for b in range(B)]
    kw = dict(trace=True) if TRACE else {}
    res = run_bass_kernel_spmd(nc, in_maps, list(range(B)), **kw)
    global LAST_RESULT
    LAST_RESULT = res
    out = res.results
    aln = np.stack([np.asarray(out[b]['o_aln']) for b in range(B)]).astype(np.float32)
    th = np.stack([np.asarray(out[b]['o_th']) for b in range(B)]).astype(np.float32)
    A = np.stack([np.asarray(out[b]['o_a']) for b in range(B)]).astype(np.float32)
    return aln, th, A
